# revision 5
# baseline (speedup 1.0000x reference)
"""Trainium2 kernel: y = relu((x - pb) @ W + b) with per-row top-K threshold masking.

Data-parallel over rows across 8 cores (per spec sharding hint). Matmul uses a
1.5-cycle/row decomposition: one fp16 main pass (xh @ wh) plus one fp8e4m3
DoubleRow pass computing both cross terms (xh @ wl + xl @ wh) in a single
0.5-cycle/row instruction (two packed 128-contraction slots). All four product
terms are pre-scaled on the host to a common 2^22 product scale so they
accumulate into one PSUM bank; a single ACT relu-copy with scale 2^-22
recovers natural-scale activations (~1e-5 relative accuracy). Per-row K-th
largest is found by a count binary search on DVE/ACT (as in the baseline),
then acts are masked and stored.
"""
import sys
sys.path.insert(0, "/opt/trn_rl_repo")

import numpy as np
import ml_dtypes
import concourse.bass as bass
import concourse.bacc as bacc
import concourse.mybir as mybir
from concourse.tile import TileContext
from contextlib import ExitStack

F32 = mybir.dt.float32
F16 = mybir.dt.float16
FP8 = mybir.dt.float8e4
E4 = ml_dtypes.float8_e4m3

# full problem dims (hardcoded; kernel.py must be self-contained)
B_FULL, D_IN, N_FEAT, K_TOP = 16384, 4096, 4096, 128
N_CORES = 8

# product scale 2^22 = (xh*2^2)(wh*2^20) = (xh*2^5)(wl*2^17) = (xl*2^17)(wh*2^5)
SX16, SW16 = 2.0 ** 2, 2.0 ** 20
SX8H, SW8L = 2.0 ** 5, 2.0 ** 17
SX8L, SW8H = 2.0 ** 17, 2.0 ** 5
SOUT = 2.0 ** -22
FP8MAX = 240.0


def build_nc(B_core, D, F, K, n_iters=18, G=4, fb=512, repeat=1):
    assert B_core % 128 == 0 and D % 128 == 0 and F % (2 * fb) == 0
    nc = bacc.Bacc("TRN2", target_bir_lowering=False, debug=True)
    xh16t = nc.dram_tensor("xh16t", [D, B_core], F16, kind="ExternalInput")
    x8t = nc.dram_tensor("x8t", [D, 2, B_core], FP8, kind="ExternalInput")
    w16 = nc.dram_tensor("w16", [D, F], F16, kind="ExternalInput")
    w8 = nc.dram_tensor("w8", [D, 2, F], FP8, kind="ExternalInput")
    out = nc.dram_tensor("out", [B_core, F], F32, kind="ExternalOutput")

    n_rb = B_core // 128
    n_db = D // 128
    n_fp = F // (2 * fb)
    groups = [list(range(i, min(i + G, n_rb))) for i in range(0, n_rb, G)]
    DR = mybir.MatmulPerfMode.DoubleRow

    with TileContext(nc) as tc:
        ctx = ExitStack()
        xh_pool = ctx.enter_context(tc.tile_pool(name="xh", bufs=n_db + 4))
        x8_pool = ctx.enter_context(tc.tile_pool(name="x8", bufs=n_db + 4))
        w16_pool = ctx.enter_context(tc.tile_pool(name="w16", bufs=6))
        w8_pool = ctx.enter_context(tc.tile_pool(name="w8", bufs=6))
        acts_pool = ctx.enter_context(tc.tile_pool(name="acts", bufs=G + 1))
        scr_pool = ctx.enter_context(tc.tile_pool(name="scr", bufs=1))
        scra_pool = ctx.enter_context(tc.tile_pool(name="scra", bufs=1))
        sm_pool = ctx.enter_context(tc.tile_pool(name="sm", bufs=12))
        mm_pool = ctx.enter_context(tc.tile_pool(name="mm", bufs=8, space="PSUM"))

        for rep in range(repeat):
            for sup in groups:
                ns = len(sup)
                gsl = slice(sup[0] * 128, (sup[-1] + 1) * 128)
                xh_t, x8_t = [], []
                for db in range(n_db):
                    dsl = slice(db * 128, (db + 1) * 128)
                    xh = xh_pool.tile([128, ns * 128], F16, tag="xh")
                    nc.sync.dma_start(out=xh[:], in_=xh16t[dsl, gsl])
                    x8 = x8_pool.tile([128, 2, ns * 128], FP8, tag="x8")
                    nc.sync.dma_start(out=x8[:], in_=x8t[dsl, :, gsl])
                    xh_t.append(xh)
                    x8_t.append(x8)

                acts = [acts_pool.tile([128, F], F32, tag="acts", name=f"acts{_i}")
                        for _i in range(ns)]
                for fp in range(n_fp):
                    f0 = fp * 2 * fb
                    pms = [[mm_pool.tile([128, fb], F32, tag="mm",
                                         name=f"pm{_i}_{_j}")
                            for _j in range(2)] for _i in range(ns)]
                    for db in range(n_db):
                        dsl = slice(db * 128, (db + 1) * 128)
                        wt = w16_pool.tile([128, 2 * fb], F16, tag="w16")
                        nc.sync.dma_start(out=wt[:], in_=w16[dsl, f0:f0 + 2 * fb])
                        w8t_ = w8_pool.tile([128, 2, 2 * fb], FP8, tag="w8")
                        nc.sync.dma_start(out=w8t_[:], in_=w8[dsl, :, f0:f0 + 2 * fb])
                        first, last = db == 0, db == n_db - 1
                        for i in range(ns):
                            xsl = slice(i * 128, (i + 1) * 128)
                            for j in range(2):
                                nc.tensor.matmul(pms[i][j][:], xh_t[db][:, xsl],
                                                 wt[:, j * fb:(j + 1) * fb],
                                                 start=first, stop=False)
                            for j in range(2):
                                nc.tensor.matmul(pms[i][j][:], x8_t[db][:, :, xsl],
                                                 w8t_[:, :, j * fb:(j + 1) * fb],
                                                 start=False, stop=last,
                                                 perf_mode=DR)
                    for i in range(ns):
                        for j in range(2):
                            nc.scalar.activation(
                                acts[i][:, f0 + j * fb:f0 + (j + 1) * fb],
                                pms[i][j][:],
                                mybir.ActivationFunctionType.Relu, scale=SOUT)

                # ---- per-row K-th largest via count binary search ----
                # invariant: count(acts >= lo) >= K, count(acts >= lo + wdt) < K
                lo = sm_pool.tile([128, ns], F32, tag="sm")
                nc.vector.memset(lo[:], 0.0)
                wdt = sm_pool.tile([128, ns], F32, tag="sm")
                for i in range(ns):
                    nc.vector.reduce_max(out=wdt[:, i:i + 1], in_=acts[i][:],
                                         axis=mybir.AxisListType.X)
                nc.vector.tensor_scalar(wdt[:], wdt[:], 1.0001, 1e-20,
                                        op0=mybir.AluOpType.mult,
                                        op1=mybir.AluOpType.add)
                mid = sm_pool.tile([128, ns], F32, tag="sm")
                nc.vector.tensor_scalar_mul(mid[:], wdt[:], 0.5)
                cnt = sm_pool.tile([128, ns], F32, tag="sm")
                tgw = sm_pool.tile([128, ns], F32, tag="sm")
                for it in range(n_iters):
                    for i in range(ns):
                        if i % 2 == 0:
                            # DVE: exact count of acts >= mid
                            scr = scr_pool.tile([128, F], FP8, tag="scr")
                            nc.vector.tensor_scalar(scr[:], acts[i][:],
                                                    mid[:, i:i + 1], None,
                                                    op0=mybir.AluOpType.is_ge,
                                                    op1=mybir.AluOpType.add,
                                                    accum_out=cnt[:, i:i + 1])
                        else:
                            # ACT: S' = sum(sign(mid - a)); count_eff=(F-S')/2
                            scr2 = scra_pool.tile([128, F], FP8, tag="scra")
                            nc.scalar.activation(scr2[:], acts[i][:],
                                                 mybir.ActivationFunctionType.Sign,
                                                 bias=mid[:, i:i + 1], scale=-1.0,
                                                 accum_out=cnt[:, i:i + 1])
                            nc.vector.tensor_scalar(cnt[:, i:i + 1],
                                                    cnt[:, i:i + 1],
                                                    -0.5, float(F) / 2.0,
                                                    op0=mybir.AluOpType.mult,
                                                    op1=mybir.AluOpType.add)
                    # wdt *= 0.5 ; lo += (cnt >= K - 0.75) * wdt ; mid = lo + wdt/2
                    nc.vector.tensor_scalar_mul(wdt[:], wdt[:], 0.5)
                    nc.vector.scalar_tensor_tensor(out=tgw[:], in0=cnt[:],
                                                   scalar=float(K) - 0.75,
                                                   in1=wdt[:],
                                                   op0=mybir.AluOpType.is_ge,
                                                   op1=mybir.AluOpType.mult)
                    nc.vector.tensor_tensor(out=lo[:], in0=lo[:], in1=tgw[:],
                                            op=mybir.AluOpType.add)
                    if it != n_iters - 1:
                        nc.vector.scalar_tensor_tensor(out=mid[:], in0=wdt[:],
                                                       scalar=0.5, in1=lo[:],
                                                       op0=mybir.AluOpType.mult,
                                                       op1=mybir.AluOpType.add)
                # ---- apply mask: out = acts * (acts >= lo) ----
                for i, r in enumerate(sup):
                    nc.vector.scalar_tensor_tensor(out=acts[i][:], in0=acts[i][:],
                                                   scalar=lo[:, i:i + 1],
                                                   in1=acts[i][:],
                                                   op0=mybir.AluOpType.is_ge,
                                                   op1=mybir.AluOpType.mult)
                    nc.sync.dma_start(out=out[r * 128:(r + 1) * 128, :],
                                      in_=acts[i][:])
        ctx.close()

    nc.finalize()
    return nc


def host_prep(x, W):
    """Split/scale/transpose inputs on the host. Returns per-core in_maps."""
    B, D = x.shape
    F = W.shape[1]
    xh = x.astype(np.float16)
    xl = x - xh.astype(np.float32)
    wh = W.astype(np.float16)
    wl = W - wh.astype(np.float32)

    xh16t = np.ascontiguousarray((xh * np.float16(SX16)).T)
    xh32t = xh.astype(np.float32).T
    x8t = np.empty((D, 2, B), dtype=E4)
    x8t[:, 0, :] = np.clip(xh32t * SX8H, -FP8MAX, FP8MAX).astype(E4)
    x8t[:, 1, :] = np.clip(xl.T * SX8L, -FP8MAX, FP8MAX).astype(E4)
    w16 = (wh.astype(np.float32) * SW16).astype(np.float16)
    w8 = np.empty((D, 2, F), dtype=E4)
    w8[:, 0, :] = np.clip(wl * SW8L, -FP8MAX, FP8MAX).astype(E4)
    w8[:, 1, :] = np.clip(wh.astype(np.float32) * SW8H, -FP8MAX, FP8MAX).astype(E4)

    B_core = B // N_CORES
    in_maps = []
    for ci in range(N_CORES):
        csl = slice(ci * B_core, (ci + 1) * B_core)
        in_maps.append({
            "xh16t": np.ascontiguousarray(xh16t[:, csl]),
            "x8t": np.ascontiguousarray(x8t[:, :, csl]),
            "w16": w16,
            "w8": w8,
        })
    return in_maps


_NC_CACHE = {}


def _get_nc(key):
    if key not in _NC_CACHE:
        _NC_CACHE[key] = build_nc(*key)
    return _NC_CACHE[key]


def kernel(x, preencoder_bias, W_enc, b_enc):
    from concourse.bass_utils import run_bass_kernel_spmd
    x = np.asarray(x, dtype=np.float32)
    W = np.asarray(W_enc, dtype=np.float32)
    pb = np.asarray(preencoder_bias, dtype=np.float32)
    b = np.asarray(b_enc, dtype=np.float32)

    B, D = x.shape
    F = W.shape[1]
    assert (B, D, F) == (B_FULL, D_IN, N_FEAT)
    # fold biases: (x - pb) @ W + b == x @ W + (b - pb @ W)
    c = (b - pb @ W).astype(np.float32)
    if np.any(c != 0.0):
        # exact-enough: augment the contraction with one extra 128-block where
        # x_aug[:, D] = 2^13 and W_aug[D, :] = c * 2^-13 (rest zeros); the
        # fp8 slot-0 saturation for this column only loses a 2^-11-relative
        # correction of c.
        pad = 128
        x_aug = np.zeros((B, D + pad), dtype=np.float32)
        x_aug[:, :D] = x
        x_aug[:, D] = 8192.0
        W_aug = np.zeros((D + pad, F), dtype=np.float32)
        W_aug[:D] = W
        W_aug[D] = c / 8192.0
        x, W, D = x_aug, W_aug, D + pad

    B_core = B // N_CORES
    nc = _get_nc((B_core, D, F, K_TOP))
    in_maps = host_prep(x, W)
    res = run_bass_kernel_spmd(nc, in_maps, core_ids=list(range(N_CORES)))
    return np.concatenate([res.results[i]["out"] for i in range(N_CORES)], axis=0)
